# revision 1
# baseline (speedup 1.0000x reference)
"""Data-parallel Trainium2 kernel for nn_Attention_18923625906513.

Shards batch B=16 across 8 NeuronCores (2 per core); all weights are
replicated. Each core runs the full ConvEncoder -> ConvBNReLU -> 3x MHA
pipeline on its batch shard; outputs are gathered to full shape.
"""

import numpy as np
import jax
import jax.numpy as jnp

DIM = 384
HEADS = 6
HEAD_DIM = DIM // HEADS
SCALE = HEAD_DIM ** -0.5
B = 16
HW = 56
N = HW * HW
NUM_TOKENS = 49
NCORES = 8

_WEIGHT_NAMES = (
    'we_dw_w', 'we_dw_b', 'we_ln_g', 'we_ln_b', 'we_pw1_w', 'we_pw1_b',
    'we_pw2_w', 'we_pw2_b', 'we_gamma', 'cb_conv_w', 'cb_ln_g', 'cb_ln_b',
    'ga_qkv_w', 'ug_kv_w', 'ug_q_w', 'gb_kv_w', 'gb_q_w', 'proj_w', 'proj_b',
)


def _layernorm(x, g, b, eps):
    m = x.mean(-1, keepdims=True)
    v = ((x - m) ** 2).mean(-1, keepdims=True)
    return (x - m) * jax.lax.rsqrt(v + eps) * g + b


def _avgpool2(x):
    Bx, C, H, W = x.shape
    return x.reshape(Bx, C, H // 2, 2, W // 2, 2).mean(axis=(3, 5))


def _mha(q, k, v):
    Bx, Nq, C = q.shape
    Nk = k.shape[1]
    qh = q.reshape(Bx, Nq, HEADS, HEAD_DIM).transpose(0, 2, 1, 3) * SCALE
    kh = k.reshape(Bx, Nk, HEADS, HEAD_DIM).transpose(0, 2, 1, 3)
    vh = v.reshape(Bx, Nk, HEADS, HEAD_DIM).transpose(0, 2, 1, 3)
    attn = jax.nn.softmax(jnp.einsum('bhqd,bhkd->bhqk', qh, kh), axis=-1)
    out = jnp.einsum('bhqk,bhkd->bhqd', attn, vh)
    return out.transpose(0, 2, 1, 3).reshape(Bx, Nq, C)


def _forward(x, global_token, we_dw_w, we_dw_b, we_ln_g, we_ln_b, we_pw1_w,
             we_pw1_b, we_pw2_w, we_pw2_b, we_gamma, cb_conv_w, cb_ln_g,
             cb_ln_b, ga_qkv_w, ug_kv_w, ug_q_w, gb_kv_w, gb_q_w, proj_w,
             proj_b):
    Bx = x.shape[0]
    x_origin = x
    xc = x.reshape(Bx, HW, HW, DIM).transpose(0, 3, 1, 2)
    res = xc
    y = jax.lax.conv_general_dilated(
        xc, we_dw_w, (1, 1), 'SAME', feature_group_count=DIM,
        dimension_numbers=('NCHW', 'OIHW', 'NCHW')) + we_dw_b[None, :, None, None]
    y = y.transpose(0, 2, 3, 1)
    y = _layernorm(y, we_ln_g, we_ln_b, 1e-6)
    y = jax.nn.gelu(y @ we_pw1_w + we_pw1_b, approximate=False)
    y = (y @ we_pw2_w + we_pw2_b) * we_gamma
    xc = res + y.transpose(0, 3, 1, 2)
    x_local = xc.reshape(Bx, DIM, N).transpose(0, 2, 1)
    xc = _avgpool2(xc)
    y = jax.lax.conv_general_dilated(
        xc, cb_conv_w, (1, 1), 'SAME',
        dimension_numbers=('NCHW', 'OIHW', 'NCHW'))
    y = _layernorm(y.transpose(0, 2, 3, 1), cb_ln_g, cb_ln_b, 1e-5)
    xc = jax.nn.relu(y).transpose(0, 3, 1, 2)
    xc = _avgpool2(xc)
    xd = xc.reshape(Bx, DIM, -1).transpose(0, 2, 1)
    q, k, v = jnp.split(xd @ ga_qkv_w, 3, axis=-1)
    x_ds = _mha(q, k, v)
    k2, v2 = jnp.split(x_ds @ ug_kv_w, 2, axis=-1)
    gt = _mha(global_token @ ug_q_w, k2, v2)
    k3, v3 = jnp.split(gt @ gb_kv_w, 2, axis=-1)
    x_global = _mha(x_origin @ gb_q_w, k3, v3)
    x_out = (x_local + x_global) @ proj_w + proj_b
    return x_out, gt


_in_axes = (0, 0) + (None,) * len(_WEIGHT_NAMES)
_pforward = jax.pmap(_forward, in_axes=_in_axes)


def kernel(**inputs):
    x = np.asarray(inputs['x'], dtype=np.float32)
    gt = np.asarray(inputs['global_token'], dtype=np.float32)
    per = B // NCORES
    xs = x.reshape(NCORES, per, N, DIM)
    gts = gt.reshape(NCORES, per, NUM_TOKENS, DIM)
    weights = [np.asarray(inputs[n], dtype=np.float32) for n in _WEIGHT_NAMES]
    x_out, gt_out = _pforward(xs, gts, *weights)
    x_out = np.asarray(x_out).reshape(B, N, DIM)
    gt_out = np.asarray(gt_out).reshape(B, NUM_TOKENS, DIM)
    return x_out, gt_out
